# revision 39
# baseline (speedup 1.0000x reference)
"""8-core Trainium2 (Bass/Tile) kernel for nn_CrossAttention.

Sharding: pure data parallelism - batch B=8, one batch element per
NeuronCore. Each core runs the full pipeline (LayerNorm on x/context,
QKV projections, 16-head attention with relative position bias,
output projection) for its element; the host gathers the 8 outputs.

Host-side prep (constant transforms of the inputs):
  - gamma folded into W{q,k,v} rows; beta folded into additive bias
    vectors beta@W{q,k,v}.
  - relative_position_bias b shipped as exp(b)^T in fp16; the kernel
    computes softmax numerators as exp(s)*exp(b) (no max subtraction:
    scores are O(10) so everything fits fp16 range).
  - weights cast to fp16 (matmuls run fp16 x fp16 -> fp32 psum).

Differences vs the earlier version (the big perf win): the attn@v
matmuls are FLIPPED - lhsT = eh tile ([128 k, 128 q]), rhs = v for one
head ([128 k, 64 d]) -> psum [128 q, 64 d], 8 kt accumulation steps.
Charged cost on the PE is output-rows streamed, so per head this is
8kt x 8qt x 64 = 4k rows instead of 8kt x 2 x 512 = 8k rows.  Rowsums
come from extra N=1 matmuls against a ones column.  The normalized
[q, hd] result is transposed back to [hd, q] with PE transposes so the
output projection is unchanged.

The whole head loop is software-pipelined at kt-slot granularity: each
slot is one scores matmul pair (the ACT exp of a [128,1024] psum tile
paces the loop at ~1.04us/slot) plus filler PE work (next chunk's k/q
projection matmuls, the trailing head's attn@v parts, ao transposes)
so the PE never waits on the exp chain.

PSUM budget (8 banks): scores 2 bufs x [128,1024]f32 (4), k/q-proj
halves + transposes 2 bufs x 2KB slots (2), attnv data+sums
[128,1024]f32 (2; data in bank0 as 8qt x 64, rowsums in bank1).
"""

import numpy as np

import concourse.bass as bass
import concourse.bacc as bacc
import concourse.tile as tile
from concourse import mybir
from concourse.masks import make_identity
from concourse.bass_utils import run_bass_kernel_spmd

f32 = mybir.dt.float32
f16 = mybir.dt.float16
AF = mybir.ActivationFunctionType
ALU = mybir.AluOpType

N = 1024
D = 1024
H = 16
NT = 8
KC = 8
EPS = 1e-5
SM_SCALE = 0.125
N_CORES = 8


def _body(tc, nc, x_in, c_in, wq_in, wk_in, wv_in, wo_in, bqkv_in, bo_in,
          ebt_in, out_d):
    with (
        tc.tile_pool(name="consts", bufs=1) as consts,
        tc.tile_pool(name="big", bufs=1) as big,
        tc.tile_pool(name="pacc", bufs=2, space="PSUM") as pacc,
        tc.tile_pool(name="pproj", bufs=2, space="PSUM") as pproj,
        tc.tile_pool(name="pas", bufs=1, space="PSUM") as pas_pool,
    ):
        ident = consts.tile([128, 128], f16)
        make_identity(nc, ident[:])
        eps_t = consts.tile([128, 1], f32)
        nc.vector.memset(eps_t[:], EPS)
        ones1 = consts.tile([1, 128], f16)
        nc.vector.memset(ones1[:], 1.0)
        scratch = consts.tile([128, 1], f32)
        bq_t = consts.tile([128, KC], f32)
        bk_t = consts.tile([128, KC], f32)
        bv_ap = bqkv_in[2, :]
        bv_b = consts.tile([128, D], f16)
        nc.gpsimd.dma_start(out=bv_b[:], in_=bass.AP(
            tensor=bv_ap.tensor, offset=bv_ap.offset,
            ap=[[0, 128]] + list(bv_ap.ap)))
        bo_ap = bo_in[:]
        bo_b = consts.tile([128, D], f16)

        xnT = big.tile([128, KC, N], f16)
        cnT = big.tile([128, KC, N], f16)
        v_aug = big.tile([128, NT, H, 65], f16)
        aoT = big.tile([128, KC, N], f16)
        wq16 = big.tile([128, KC, D], f16)
        wk16 = big.tile([128, KC, D], f16)
        wo16 = big.tile([128, KC, D], f16)
        nc.gpsimd.memset(v_aug[:, :, :, 0:1], 1.0)
        v_flat = v_aug[:].rearrange("p a h c -> p a (h c)")

        def ln_tile(pool_a, dst_T, src_dram, t, copy_eng, split_dma=False):
            """LayerNorm (no gamma/beta) + PE transpose of token tile t."""
            xt = pool_a.tile([128, D], f16, tag="xt")
            rows = src_dram[t * 128:(t + 1) * 128, :]
            if split_dma:
                nc.sync.dma_start(out=xt[:, 0:512], in_=rows[:, 0:512])
                nc.sync.dma_start(out=xt[:, 512:1024], in_=rows[:, 512:1024])
            else:
                nc.sync.dma_start(out=xt[:], in_=rows)
            stats = pool_a.tile([128, 2, 6], f32, tag="stats")
            xv = xt[:].rearrange("p (a b) -> p a b", a=2)
            nc.vector.bn_stats(out=stats[:, 0, :], in_=xv[:, 0, :])
            nc.vector.bn_stats(out=stats[:, 1, :], in_=xv[:, 1, :])
            mv = pool_a.tile([128, 2], f32, tag="mv")
            nc.vector.bn_aggr(out=mv[:], in_=stats[:])
            sd = pool_a.tile([128, 2], f32, tag="sd")
            nc.scalar.activation(out=sd[:, 0:1], in_=mv[:, 1:2], func=AF.Sqrt,
                                 bias=eps_t[:], scale=1.0)
            nc.vector.reciprocal(out=sd[:, 1:2], in_=sd[:, 0:1])
            xn16 = pool_a.tile([128, D], f16, tag="xn16")
            ts_eng = nc.vector if t == 0 else nc.gpsimd
            ts_eng.tensor_scalar(out=xn16[:], in0=xt[:], scalar1=mv[:, 0:1],
                                 scalar2=sd[:, 1:2], op0=ALU.subtract,
                                 op1=ALU.mult)
            ptr = pproj.tile([128, KC, 128], f16, tag="proj")
            for c in range(KC):
                nc.tensor.transpose(ptr[:, c, :], xn16[:, c * 128:(c + 1) * 128],
                                    ident[:])
            dst = dst_T[:, :, t * 128:(t + 1) * 128]
            if copy_eng == "act":
                nc.scalar.copy(out=dst, in_=ptr[:])
            else:
                nc.vector.tensor_scalar_mul(
                    out=dst, in0=ptr[:], scalar1=1.0)

        def emit_vproj(t, wv16):
            pv = pacc.tile([128, N], f32, tag="acc")
            for kc in range(KC):
                for nh in range(2):
                    nc.tensor.matmul(
                        pv[:, nh * 512:(nh + 1) * 512],
                        cnT[:, kc, t * 128:(t + 1) * 128],
                        wv16[:, kc, nh * 512:(nh + 1) * 512],
                        start=(kc == 0), stop=(kc == KC - 1))
            nc.vector.tensor_add(
                out=v_aug[:, t, :, 1:65],
                in0=pv[:].rearrange("p (h d) -> p h d", h=H),
                in1=bv_b[:].rearrange("p (h d) -> p h d", h=H))

        ebh_tiles = {}

        def emit_ebh(h, pebt):
            if h >= H:
                return
            for half in range(2):
                t = pebt.tile([128, 4, N], f16, tag="ebt")
                nc.sync.dma_start(
                    out=t[:],
                    in_=ebt_in[h, half * 512:(half + 1) * 512, :].rearrange(
                        "(a p) q -> p a q", p=128))
                ebh_tiles[(h, half)] = t

        # ---------------- prologue ----------------
        # ctx/x LN tiles interleaved; DMA issue order is chosen so the
        # serial DMA resource feeds each consumer just in time.
        with tc.tile_pool(name="pa", bufs=6) as pool_a, \
             tc.tile_pool(name="pwv", bufs=1) as pwv:
            wv16 = pwv.tile([128, KC, D], f16)
            wv_r = wv_in.rearrange("(a p) m -> p a m", p=128)
            ln_tile(pool_a, cnT, c_in, 0, "act", split_dma=True)
            ln_tile(pool_a, xnT, x_in, 0, "act")
            nc.sync.dma_start(out=bq_t[:], in_=bqkv_in[0, :].rearrange(
                "(m p) -> p m", p=128))
            nc.sync.dma_start(out=bk_t[:], in_=bqkv_in[1, :].rearrange(
                "(m p) -> p m", p=128))
            for t in range(1, NT):
                ln_tile(pool_a, cnT, c_in, t, "act")
                if t in (1, 2):
                    h = t - 1
                    nc.sync.dma_start(out=wv16[:, 4 * h:4 * h + 4, :],
                                      in_=wv_r[:, 4 * h:4 * h + 4, :])
                if t == 3:
                    nc.sync.dma_start(out=wk16[:], in_=wk_in.rearrange(
                        "(a p) m -> p a m", p=128))
                if t == 5:
                    nc.sync.dma_start(out=wq16[:], in_=wq_in.rearrange(
                        "(a p) m -> p a m", p=128))
                ln_tile(pool_a, xnT, x_in, t, "act")
                if t >= 2:
                    emit_vproj(t - 2, wv16)
            emit_vproj(NT - 2, wv16)
            emit_vproj(NT - 1, wv16)

        # chunk-granular k/q tiles: [dout-in-chunk (128), token (1024)]
        with tc.tile_pool(name="pkq", bufs=4) as pkq, \
             tc.tile_pool(name="pc", bufs=3) as pc, \
             tc.tile_pool(name="pebt", bufs=3) as pebt, \
             tc.tile_pool(name="pao", bufs=2) as pao, \
             tc.tile_pool(name="prec", bufs=2) as prec:

            kq_tiles = {}

            def proj_mm_half(which, c, half, kc_pair):
                """Two accumulation matmuls of the (c, half) proj into pproj."""
                key = ("k" if which == "k" else "q", c, half)
                if key not in kq_tiles and kc_pair == 0:
                    kq_tiles[key] = pproj.tile([128, 512], f32, tag="proj",
                                               name=f"p{which}{c}h{half}")
                ph = kq_tiles[key]
                w = wk16 if which == "k" else wq16
                src = cnT if which == "k" else xnT
                for kc in (2 * kc_pair, 2 * kc_pair + 1):
                    nc.tensor.matmul(
                        ph[:], w[:, kc, c * 128:(c + 1) * 128],
                        src[:, kc, half * 512:(half + 1) * 512],
                        start=(kc == 0), stop=(kc == KC - 1))

            def proj_bias(which, c, half, eng="dve"):
                key = ("k" if which == "k" else "q", c, half)
                ph = kq_tiles.pop(key)
                dkey = (which, c)
                if dkey not in kq_tiles:
                    kq_tiles[dkey] = pkq.tile([128, N], f16, tag="kq",
                                              name=f"{which}T{c}")
                bias = bk_t if which == "k" else bq_t
                dst = kq_tiles[dkey][:, half * 512:(half + 1) * 512]
                if eng == "act":
                    nc.scalar.add(out=dst, in_=ph[:], add=bias[:, c:c + 1])
                else:
                    nc.vector.tensor_scalar(
                        out=dst, in0=ph[:], scalar1=bias[:, c:c + 1],
                        scalar2=None, op0=ALU.add)

            def scores_slot(h, kt, eh_t):
                """One kt tile of scores + exp + ebt multiply."""
                ch, r0 = h // 2, (h % 2) * 64
                kTc = kq_tiles[("k", ch)]
                qTc = kq_tiles[("q", ch)]
                ps = pacc.tile([128, N], f32, tag="acc")
                for nh in range(2):
                    nc.tensor.matmul(
                        ps[:, nh * 512:(nh + 1) * 512],
                        kTc[r0:r0 + 64, kt * 128:(kt + 1) * 128],
                        qTc[r0:r0 + 64, nh * 512:(nh + 1) * 512],
                        start=True, stop=True)
                nc.scalar.activation(out=eh_t[:, kt, :], in_=ps[:],
                                     func=AF.Exp, scale=SM_SCALE)
                eng = nc.gpsimd if (kt in (3, 6) and h < H - 2) else nc.vector
                eng.tensor_mul(out=eh_t[:, kt, :], in0=eh_t[:, kt, :],
                               in1=ebh_tiles[(h, kt // 4)][:, kt % 4, :])

            def attnv_part(h, qt, eh_t, pas):
                """attn@v for one qt block of head h: 8 data + 8 sum mms."""
                for kt in range(NT):
                    nc.tensor.matmul(
                        pas[:, qt * 64:(qt + 1) * 64],
                        eh_t[:, kt, qt * 128:(qt + 1) * 128],
                        v_flat[:, kt, h * 65 + 1:h * 65 + 65],
                        start=(kt == 0), stop=(kt == NT - 1))
                for kt in range(NT):
                    nc.tensor.matmul(
                        pas[:, 512 + qt:513 + qt],
                        eh_t[:, kt, qt * 128:(qt + 1) * 128],
                        v_flat[:, kt, h * 65:h * 65 + 1],
                        start=(kt == 0), stop=(kt == NT - 1))

            def emit_norm(h, pas, ao_t):
                """reciprocal of rowsums + broadcast-normalize into ao."""
                rec = prec.tile([128, 8], f32, tag="rec")
                nc.vector.reciprocal(out=rec[:], in_=pas[:, 512:520])
                rec_b = bass.AP(tensor=rec.tensor, offset=rec.offset,
                                ap=[[8, 128], [1, 8], [0, 64]])
                nc.vector.tensor_mul(
                    out=ao_t[:, :, h % 2, :],
                    in0=pas[:, 0:512].rearrange("p (a b) -> p a b", a=8),
                    in1=rec_b)

            def emit_transpose(ch, ao_t):
                """ao [q, hd] -> aoT [hd, q] for one chunk."""
                ptr = pproj.tile([128, NT, 128], f16, tag="proj",
                                 name=f"ptr{ch}")
                for qt in range(NT):
                    nc.tensor.transpose(
                        ptr[:, qt, :],
                        ao_t[:, qt, :, :].rearrange("p a b -> p (a b)"),
                        ident[:])
                nc.vector.tensor_scalar_mul(
                    out=aoT[:, ch, :],
                    in0=ptr[:].rearrange("p a b -> p (a b)"), scalar1=1.0)

            def make_proj_units(which, c, bias_eng="dve"):
                units = []
                for half in range(2):
                    for p in range(4):
                        def u(which=which, c=c, half=half, p=p):
                            proj_mm_half(which, c, half, p)
                            if p == 3:
                                proj_bias(which, c, half, bias_eng)
                        units.append(u)
                return units

            # prologue part 2: k/q projections for chunk 0 (burst)
            emit_ebh(0, pebt)
            emit_ebh(1, pebt)
            for u in (make_proj_units("k", 0, "act")
                      + make_proj_units("q", 0, "act")):
                u()
            # preload the Exp ACT table while the PE runs the bursts above
            nc.scalar.activation(out=scratch[:], in_=eps_t[:], func=AF.Exp,
                                 scale=1.0)

            # ---------------- head loop ----------------
            # state carried between chunks
            prev = {}  # h -> (eh_t, pas, ao_t) for heads with pending work

            eh_prev = None     # eh of head 2c-1
            pas_prev = None    # pas tile of head 2c-1 (attnv in flight)
            spill = None       # (h, eh, pas, ao_t, qts) attnv spill from 2c-2
            ao_prev = None     # ao tile of chunk c-1
            ao_cur = None

            for c in range(KC):
                h0, h1 = 2 * c, 2 * c + 1
                emit_ebh(h0 + 2, pebt)
                if c == 5:
                    nc.sync.dma_start(out=wo16[:], in_=wo_in.rearrange(
                        "(a p) m -> p a m", p=128))
                if c == 6:
                    nc.gpsimd.dma_start(out=bo_b[:], in_=bass.AP(
                        tensor=bo_ap.tensor, offset=bo_ap.offset,
                        ap=[[0, 128]] + list(bo_ap.ap)))
                ao_last, ao_cur = ao_cur, pao.tile([128, NT, 2, 64], f16,
                                                   tag="ao", name=f"ao{c}")

                eh0 = pc.tile([128, NT, N], f16, tag="et", name=f"eh{h0}")
                if c == 0:
                    units_k = make_proj_units("k", 1) + make_proj_units("q", 1)
                    k_fill = [2, 2, 2, 2, 2, 2, 2, 2]
                elif c < KC - 1:
                    units_k = make_proj_units("k", c + 1)
                    k_fill = [2, 2, 1, 1, 1, 1, 0, 0]
                else:
                    units_k, k_fill = [], [0] * 8
                # --- h0 phase: 8 slots ---
                for kt in range(NT):
                    if kt > 0:
                        scores_slot(h0, kt, eh0)
                    for _ in range(k_fill[kt]):
                        if units_k:
                            units_k.pop(0)()
                    if kt == 0:
                        scores_slot(h0, kt, eh0)
                    # spill: finish attnv of head 2c-2 (qt 6,7)
                    if spill is not None and kt < len(spill[4]):
                        sh, seh, spas, sao, qts = spill
                        attnv_part(sh, qts[kt], seh, spas)
                        if qts[kt] == NT - 1:
                            emit_norm(sh, spas, sao)
                    # attnv of head 2c-1, qt 0..5 on slots 2..7
                    if eh_prev is not None and kt >= 2:
                        if kt == 2:
                            pas_prev = pas_pool.tile([128, N], f32, tag="as",
                                                     name=f"pas{h0 - 1}")
                        attnv_part(h0 - 1, kt - 2, eh_prev, pas_prev)
                spill = None

                emit_ebh(h1 + 2, pebt)
                eh1 = pc.tile([128, NT, N], f16, tag="et", name=f"eh{h1}")
                # --- h1 phase: 8 slots ---
                units_q = make_proj_units("q", c + 1) if 0 < c < KC - 1 else []
                q_fill = [2, 2, 1, 1, 1, 1, 0, 0]
                for kt in range(NT):
                    if kt > 0:
                        scores_slot(h1, kt, eh1)
                    for _ in range(q_fill[kt]):
                        if units_q:
                            units_q.pop(0)()
                    if kt == 0:
                        scores_slot(h1, kt, eh1)
                    if eh_prev is not None and kt < 2:
                        # finish attnv of head 2c-1 (qt 6,7)
                        attnv_part(h0 - 1, 6 + kt, eh_prev, pas_prev)
                        if kt == 1:
                            emit_norm(h0 - 1, pas_prev, ao_last)
                            kq_tiles.pop(("k", c - 1), None)
                            kq_tiles.pop(("q", c - 1), None)
                    if kt == 2 and c >= 1:
                        emit_transpose(c - 1, ao_last)
                    # attnv of head 2c, qt 0..5 on slots 2..7
                    if kt >= 2:
                        if kt == 2:
                            pas0 = pas_pool.tile([128, N], f32, tag="as",
                                                 name=f"pas{h0}")
                        attnv_part(h0, kt - 2, eh0, pas0)
                if c < KC - 1:
                    spill = (h0, eh0, pas0, ao_cur, (6, 7))
                else:
                    for qt in (6, 7):
                        attnv_part(h0, qt, eh0, pas0)
                    emit_norm(h0, pas0, ao_cur)
                eh_prev, pas_prev = eh1, None

            # ---------------- epilogue ----------------
            def oproj_mm(fo, m, nh, kc):
                nc.tensor.matmul(
                    fo[:, nh * 512:(nh + 1) * 512],
                    aoT[:, kc, m * 128:(m + 1) * 128],
                    wo16[:, kc, nh * 512:(nh + 1) * 512],
                    start=(kc == 0), stop=(kc == KC - 1))

            def oproj_out(fo, m, nh):
                so = pc.tile([128, N], f16, tag="so", name=f"so{m}{nh}")
                nh_sl = slice(nh * 512, (nh + 1) * 512)
                nc.vector.tensor_add(out=so[:, nh_sl], in0=fo[:, nh_sl],
                                     in1=bo_b[:, nh_sl])
                nc.sync.dma_start(out=out_d[m * 128:(m + 1) * 128, nh_sl],
                                  in_=so[:, nh_sl])

            # m0's first 7 kc-steps fill the PE while head 15's eh-multiply
            # chain drains; only kc=7 (needs the last transpose) is deferred
            fo0 = pacc.tile([128, N], f32, tag="acc", name="fo0")
            for nh in range(2):
                for kc in range(KC - 1):
                    oproj_mm(fo0, 0, nh, kc)
            # attnv + normalize for head 15
            pas15 = pas_pool.tile([128, N], f32, tag="as", name="pas15")
            for qt in range(NT):
                attnv_part(H - 1, qt, eh_prev, pas15)
            emit_norm(H - 1, pas15, ao_cur)
            emit_transpose(KC - 1, ao_cur)
            for nh in range(2):
                oproj_mm(fo0, 0, nh, KC - 1)
                oproj_out(fo0, 0, nh)

            for m in range(1, NT):
                if m % 2 == 1:
                    fo = pas_pool.tile([128, N], f32, tag="as", name=f"fo{m}")
                else:
                    fo = pacc.tile([128, N], f32, tag="acc", name=f"fo{m}")
                for nh in range(2):
                    for kc in range(KC):
                        oproj_mm(fo, m, nh, kc)
                    oproj_out(fo, m, nh)


def build():
    nc = bacc.Bacc()
    x_in = nc.declare_dram_parameter("x", [N, D], f16, isOutput=False)
    c_in = nc.declare_dram_parameter("ctx", [N, D], f16, isOutput=False)
    wq_in = nc.declare_dram_parameter("wq", [D, D], f16, isOutput=False)
    wk_in = nc.declare_dram_parameter("wk", [D, D], f16, isOutput=False)
    wv_in = nc.declare_dram_parameter("wv", [D, D], f16, isOutput=False)
    wo_in = nc.declare_dram_parameter("wo", [D, D], f16, isOutput=False)
    bqkv_in = nc.declare_dram_parameter("bqkv", [3, D], f32, isOutput=False)
    bo_in = nc.declare_dram_parameter("bo", [D], f32, isOutput=False)
    ebt_in = nc.declare_dram_parameter("ebt", [H, N, N], f16, isOutput=False)
    out_d = nc.declare_dram_parameter("out", [N, D], f16, isOutput=True)
    with tile.TileContext(nc) as tc:
        _body(tc, nc, x_in, c_in, wq_in, wk_in, wv_in, wo_in, bqkv_in, bo_in,
              ebt_in, out_d)
    nc.compile()
    return nc


_NC_CACHE = None


def _get_nc():
    global _NC_CACHE
    if _NC_CACHE is None:
        _NC_CACHE = build()
    return _NC_CACHE


def kernel(x, context, relative_position_bias, Wq, Wk, Wv, Wo, bo, gamma,
           beta):
    x = np.asarray(x, np.float32)
    context = np.asarray(context, np.float32)
    rpb = np.asarray(relative_position_bias, np.float32)
    Wq = np.asarray(Wq, np.float32)
    Wk = np.asarray(Wk, np.float32)
    Wv = np.asarray(Wv, np.float32)
    Wo = np.asarray(Wo, np.float32)
    bo = np.asarray(bo, np.float32)
    gamma = np.asarray(gamma, np.float32)
    beta = np.asarray(beta, np.float32)

    wq16 = (gamma[:, None] * Wq).astype(np.float16)
    wk16 = (gamma[:, None] * Wk).astype(np.float16)
    wv16 = (gamma[:, None] * Wv).astype(np.float16)
    wo16 = Wo.astype(np.float16)
    bqkv = np.stack([beta @ Wq, beta @ Wk, beta @ Wv]).astype(np.float32)
    ebt = np.exp(rpb).transpose(0, 2, 1).astype(np.float16).copy()

    shared = {
        "wq": wq16, "wk": wk16, "wv": wv16, "wo": wo16,
        "bqkv": bqkv, "bo": bo, "ebt": ebt,
    }
    in_maps = [
        {"x": np.ascontiguousarray(x[i]).astype(np.float16),
         "ctx": np.ascontiguousarray(context[i]).astype(np.float16), **shared}
        for i in range(N_CORES)
    ]

    nc = _get_nc()
    last_err = None
    for _attempt in range(3):
        try:
            res = run_bass_kernel_spmd(nc, in_maps, list(range(N_CORES)))
            break
        except Exception as e:  # transient NRT/axon exec errors
            last_err = e
    else:
        raise last_err
    return np.stack([res.results[i]["out"].astype(np.float32)
                     for i in range(N_CORES)])


# revision 40
# speedup vs baseline: 1.0040x; 1.0040x over previous
"""8-core Trainium2 (Bass/Tile) kernel for nn_CrossAttention.

Sharding: pure data parallelism - batch B=8, one batch element per
NeuronCore. Each core runs the full pipeline (LayerNorm on x/context,
QKV projections, 16-head attention with relative position bias,
output projection) for its element; the host gathers the 8 outputs.

Host-side prep (constant transforms of the inputs):
  - gamma folded into W{q,k,v} rows; beta folded into additive bias
    vectors beta@W{q,k,v}.
  - relative_position_bias b shipped as exp(b)^T in fp16; the kernel
    computes softmax numerators as exp(s)*exp(b) (no max subtraction:
    scores are O(10) so everything fits fp16 range).
  - weights cast to fp16 (matmuls run fp16 x fp16 -> fp32 psum).

Differences vs the earlier version (the big perf win): the attn@v
matmuls are FLIPPED - lhsT = eh tile ([128 k, 128 q]), rhs = v for one
head ([128 k, 64 d]) -> psum [128 q, 64 d], 8 kt accumulation steps.
Charged cost on the PE is output-rows streamed, so per head this is
8kt x 8qt x 64 = 4k rows instead of 8kt x 2 x 512 = 8k rows.  Rowsums
come from extra N=1 matmuls against a ones column.  The normalized
[q, hd] result is transposed back to [hd, q] with PE transposes so the
output projection is unchanged.

The whole head loop is software-pipelined at kt-slot granularity: each
slot is one scores matmul pair (the ACT exp of a [128,1024] psum tile
paces the loop at ~1.04us/slot) plus filler PE work (next chunk's k/q
projection matmuls, the trailing head's attn@v parts, ao transposes)
so the PE never waits on the exp chain.

PSUM budget (8 banks): scores 2 bufs x [128,1024]f32 (4), k/q-proj
halves + transposes 2 bufs x 2KB slots (2), attnv data+sums
[128,1024]f32 (2; data in bank0 as 8qt x 64, rowsums in bank1).
"""

import numpy as np

import concourse.bass as bass
import concourse.bacc as bacc
import concourse.tile as tile
from concourse import mybir
from concourse.masks import make_identity
from concourse.bass_utils import run_bass_kernel_spmd

f32 = mybir.dt.float32
f16 = mybir.dt.float16
AF = mybir.ActivationFunctionType
ALU = mybir.AluOpType

N = 1024
D = 1024
H = 16
NT = 8
KC = 8
EPS = 1e-5
SM_SCALE = 0.125
N_CORES = 8


def _body(tc, nc, x_in, c_in, wq_in, wk_in, wv_in, wo_in, bqkv_in, bo_in,
          ebt_in, out_d):
    with (
        tc.tile_pool(name="consts", bufs=1) as consts,
        tc.tile_pool(name="big", bufs=1) as big,
        tc.tile_pool(name="pacc", bufs=2, space="PSUM") as pacc,
        tc.tile_pool(name="pproj", bufs=2, space="PSUM") as pproj,
        tc.tile_pool(name="pas", bufs=1, space="PSUM") as pas_pool,
    ):
        ident = consts.tile([128, 128], f16)
        make_identity(nc, ident[:])
        eps_t = consts.tile([128, 1], f32)
        nc.vector.memset(eps_t[:], EPS)
        ones1 = consts.tile([1, 128], f16)
        nc.vector.memset(ones1[:], 1.0)
        scratch = consts.tile([128, 1], f32)
        bq_t = consts.tile([128, KC], f32)
        bk_t = consts.tile([128, KC], f32)
        bv_ap = bqkv_in[2, :]
        bv_b = consts.tile([128, D], f16)
        nc.gpsimd.dma_start(out=bv_b[:], in_=bass.AP(
            tensor=bv_ap.tensor, offset=bv_ap.offset,
            ap=[[0, 128]] + list(bv_ap.ap)))
        bo_ap = bo_in[:]
        bo_b = consts.tile([128, D], f16)

        xnT = big.tile([128, KC, N], f16)
        cnT = big.tile([128, KC, N], f16)
        v_aug = big.tile([128, NT, H, 65], f16)
        aoT = big.tile([128, KC, N], f16)
        wq16 = big.tile([128, KC, D], f16)
        wk16 = big.tile([128, KC, D], f16)
        wo16 = big.tile([128, KC, D], f16)
        nc.gpsimd.memset(v_aug[:, :, :, 0:1], 1.0)
        v_flat = v_aug[:].rearrange("p a h c -> p a (h c)")

        def ln_tile(pool_a, dst_T, src_dram, t, copy_eng, split_dma=False):
            """LayerNorm (no gamma/beta) + PE transpose of token tile t."""
            xt = pool_a.tile([128, D], f16, tag="xt")
            rows = src_dram[t * 128:(t + 1) * 128, :]
            if split_dma:
                nc.sync.dma_start(out=xt[:, 0:512], in_=rows[:, 0:512])
                nc.sync.dma_start(out=xt[:, 512:1024], in_=rows[:, 512:1024])
            else:
                nc.sync.dma_start(out=xt[:], in_=rows)
            stats = pool_a.tile([128, 2, 6], f32, tag="stats")
            xv = xt[:].rearrange("p (a b) -> p a b", a=2)
            nc.vector.bn_stats(out=stats[:, 0, :], in_=xv[:, 0, :])
            nc.vector.bn_stats(out=stats[:, 1, :], in_=xv[:, 1, :])
            mv = pool_a.tile([128, 2], f32, tag="mv")
            nc.vector.bn_aggr(out=mv[:], in_=stats[:])
            sd = pool_a.tile([128, 2], f32, tag="sd")
            nc.scalar.activation(out=sd[:, 0:1], in_=mv[:, 1:2], func=AF.Sqrt,
                                 bias=eps_t[:], scale=1.0)
            nc.vector.reciprocal(out=sd[:, 1:2], in_=sd[:, 0:1])
            xn16 = pool_a.tile([128, D], f16, tag="xn16")
            ts_eng = nc.vector if t == 0 else nc.gpsimd
            ts_eng.tensor_scalar(out=xn16[:], in0=xt[:], scalar1=mv[:, 0:1],
                                 scalar2=sd[:, 1:2], op0=ALU.subtract,
                                 op1=ALU.mult)
            ptr = pproj.tile([128, KC, 128], f16, tag="proj")
            for c in range(KC):
                nc.tensor.transpose(ptr[:, c, :], xn16[:, c * 128:(c + 1) * 128],
                                    ident[:])
            dst = dst_T[:, :, t * 128:(t + 1) * 128]
            if copy_eng == "act":
                nc.scalar.copy(out=dst, in_=ptr[:])
            else:
                nc.vector.tensor_scalar_mul(
                    out=dst, in0=ptr[:], scalar1=1.0)

        def emit_vproj(t, wv16):
            pv = pacc.tile([128, N], f32, tag="acc")
            for kc in range(KC):
                for nh in range(2):
                    nc.tensor.matmul(
                        pv[:, nh * 512:(nh + 1) * 512],
                        cnT[:, kc, t * 128:(t + 1) * 128],
                        wv16[:, kc, nh * 512:(nh + 1) * 512],
                        start=(kc == 0), stop=(kc == KC - 1))
            nc.vector.tensor_add(
                out=v_aug[:, t, :, 1:65],
                in0=pv[:].rearrange("p (h d) -> p h d", h=H),
                in1=bv_b[:].rearrange("p (h d) -> p h d", h=H))

        ebh_tiles = {}

        def emit_ebh(h, pebt):
            if h >= H:
                return
            for half in range(2):
                t = pebt.tile([128, 4, N], f16, tag="ebt")
                nc.sync.dma_start(
                    out=t[:],
                    in_=ebt_in[h, half * 512:(half + 1) * 512, :].rearrange(
                        "(a p) q -> p a q", p=128))
                ebh_tiles[(h, half)] = t

        # ---------------- prologue ----------------
        # ctx/x LN tiles interleaved; DMA issue order is chosen so the
        # serial DMA resource feeds each consumer just in time.
        with tc.tile_pool(name="pa", bufs=6) as pool_a, \
             tc.tile_pool(name="pwv", bufs=1) as pwv:
            wv16 = pwv.tile([128, KC, D], f16)
            wv_r = wv_in.rearrange("(a p) m -> p a m", p=128)
            ln_tile(pool_a, cnT, c_in, 0, "act", split_dma=True)
            ln_tile(pool_a, xnT, x_in, 0, "act")
            nc.sync.dma_start(out=bq_t[:], in_=bqkv_in[0, :].rearrange(
                "(m p) -> p m", p=128))
            nc.sync.dma_start(out=bk_t[:], in_=bqkv_in[1, :].rearrange(
                "(m p) -> p m", p=128))
            for t in range(1, NT):
                ln_tile(pool_a, cnT, c_in, t, "act")
                if t in (1, 2):
                    h = t - 1
                    nc.sync.dma_start(out=wv16[:, 4 * h:4 * h + 4, :],
                                      in_=wv_r[:, 4 * h:4 * h + 4, :])
                if t == 3:
                    nc.sync.dma_start(out=wk16[:], in_=wk_in.rearrange(
                        "(a p) m -> p a m", p=128))
                if t == 5:
                    nc.sync.dma_start(out=wq16[:], in_=wq_in.rearrange(
                        "(a p) m -> p a m", p=128))
                ln_tile(pool_a, xnT, x_in, t, "act")
                if t >= 2:
                    emit_vproj(t - 2, wv16)
            emit_vproj(NT - 2, wv16)
            emit_vproj(NT - 1, wv16)

        # chunk-granular k/q tiles: [dout-in-chunk (128), token (1024)]
        with tc.tile_pool(name="pkq", bufs=4) as pkq, \
             tc.tile_pool(name="pc", bufs=3) as pc, \
             tc.tile_pool(name="pebt", bufs=3) as pebt, \
             tc.tile_pool(name="pao", bufs=2) as pao, \
             tc.tile_pool(name="prec", bufs=2) as prec:

            kq_tiles = {}

            def proj_mm_half(which, c, half, kc_pair):
                """Two accumulation matmuls of the (c, half) proj into pproj."""
                key = ("k" if which == "k" else "q", c, half)
                if key not in kq_tiles and kc_pair == 0:
                    kq_tiles[key] = pproj.tile([128, 512], f32, tag="proj",
                                               name=f"p{which}{c}h{half}")
                ph = kq_tiles[key]
                w = wk16 if which == "k" else wq16
                src = cnT if which == "k" else xnT
                for kc in (2 * kc_pair, 2 * kc_pair + 1):
                    nc.tensor.matmul(
                        ph[:], w[:, kc, c * 128:(c + 1) * 128],
                        src[:, kc, half * 512:(half + 1) * 512],
                        start=(kc == 0), stop=(kc == KC - 1))

            def proj_bias(which, c, half, eng="dve"):
                key = ("k" if which == "k" else "q", c, half)
                ph = kq_tiles.pop(key)
                dkey = (which, c)
                if dkey not in kq_tiles:
                    kq_tiles[dkey] = pkq.tile([128, N], f16, tag="kq",
                                              name=f"{which}T{c}")
                bias = bk_t if which == "k" else bq_t
                dst = kq_tiles[dkey][:, half * 512:(half + 1) * 512]
                if eng == "act":
                    nc.scalar.add(out=dst, in_=ph[:], add=bias[:, c:c + 1])
                else:
                    nc.vector.tensor_scalar(
                        out=dst, in0=ph[:], scalar1=bias[:, c:c + 1],
                        scalar2=None, op0=ALU.add)

            def scores_slot(h, kt, eh_t):
                """One kt tile of scores + exp + ebt multiply."""
                ch, r0 = h // 2, (h % 2) * 64
                kTc = kq_tiles[("k", ch)]
                qTc = kq_tiles[("q", ch)]
                ps = pacc.tile([128, N], f32, tag="acc")
                for nh in range(2):
                    nc.tensor.matmul(
                        ps[:, nh * 512:(nh + 1) * 512],
                        kTc[r0:r0 + 64, kt * 128:(kt + 1) * 128],
                        qTc[r0:r0 + 64, nh * 512:(nh + 1) * 512],
                        start=True, stop=True)
                nc.scalar.activation(out=eh_t[:, kt, :], in_=ps[:],
                                     func=AF.Exp, scale=SM_SCALE)
                eng = nc.gpsimd if (kt in (3, 6) and h < H - 2) else nc.vector
                eng.tensor_mul(out=eh_t[:, kt, :], in0=eh_t[:, kt, :],
                               in1=ebh_tiles[(h, kt // 4)][:, kt % 4, :])

            def attnv_part(h, qt, eh_t, pas):
                """attn@v for one qt block of head h: 8 data + 8 sum mms."""
                for kt in range(NT):
                    nc.tensor.matmul(
                        pas[:, qt * 64:(qt + 1) * 64],
                        eh_t[:, kt, qt * 128:(qt + 1) * 128],
                        v_flat[:, kt, h * 65 + 1:h * 65 + 65],
                        start=(kt == 0), stop=(kt == NT - 1))
                for kt in range(NT):
                    nc.tensor.matmul(
                        pas[:, 512 + qt:513 + qt],
                        eh_t[:, kt, qt * 128:(qt + 1) * 128],
                        v_flat[:, kt, h * 65:h * 65 + 1],
                        start=(kt == 0), stop=(kt == NT - 1))

            def emit_norm(h, pas, ao_t):
                """reciprocal of rowsums + broadcast-normalize into ao."""
                rec = prec.tile([128, 8], f32, tag="rec")
                nc.vector.reciprocal(out=rec[:], in_=pas[:, 512:520])
                rec_b = bass.AP(tensor=rec.tensor, offset=rec.offset,
                                ap=[[8, 128], [1, 8], [0, 64]])
                nc.vector.tensor_mul(
                    out=ao_t[:, :, h % 2, :],
                    in0=pas[:, 0:512].rearrange("p (a b) -> p a b", a=8),
                    in1=rec_b)

            def emit_transpose(ch, ao_t):
                """ao [q, hd] -> aoT [hd, q] for one chunk."""
                ptr = pproj.tile([128, NT, 128], f16, tag="proj",
                                 name=f"ptr{ch}")
                for qt in range(NT):
                    nc.tensor.transpose(
                        ptr[:, qt, :],
                        ao_t[:, qt, :, :].rearrange("p a b -> p (a b)"),
                        ident[:])
                nc.vector.tensor_scalar_mul(
                    out=aoT[:, ch, :],
                    in0=ptr[:].rearrange("p a b -> p (a b)"), scalar1=1.0)

            def make_proj_units(which, c, bias_eng="dve"):
                units = []
                for half in range(2):
                    for p in range(4):
                        def u(which=which, c=c, half=half, p=p):
                            proj_mm_half(which, c, half, p)
                            if p == 3:
                                proj_bias(which, c, half, bias_eng)
                        units.append(u)
                return units

            # prologue part 2: k/q projections for chunk 0 (burst)
            emit_ebh(0, pebt)
            emit_ebh(1, pebt)
            for u in (make_proj_units("k", 0, "act")
                      + make_proj_units("q", 0, "act")):
                u()
            # preload the Exp ACT table while the PE runs the bursts above
            nc.scalar.activation(out=scratch[:], in_=eps_t[:], func=AF.Exp,
                                 scale=1.0)

            # ---------------- head loop ----------------
            # state carried between chunks
            prev = {}  # h -> (eh_t, pas, ao_t) for heads with pending work

            eh_prev = None     # eh of head 2c-1
            pas_prev = None    # pas tile of head 2c-1 (attnv in flight)
            spill = None       # (h, eh, pas, ao_t, qts) attnv spill from 2c-2
            ao_prev = None     # ao tile of chunk c-1
            ao_cur = None

            for c in range(KC):
                h0, h1 = 2 * c, 2 * c + 1
                emit_ebh(h0 + 2, pebt)
                if c == 5:
                    nc.sync.dma_start(out=wo16[:], in_=wo_in.rearrange(
                        "(a p) m -> p a m", p=128))
                if c == 6:
                    nc.gpsimd.dma_start(out=bo_b[:], in_=bass.AP(
                        tensor=bo_ap.tensor, offset=bo_ap.offset,
                        ap=[[0, 128]] + list(bo_ap.ap)))
                ao_last, ao_cur = ao_cur, pao.tile([128, NT, 2, 64], f16,
                                                   tag="ao", name=f"ao{c}")

                eh0 = pc.tile([128, NT, N], f16, tag="et", name=f"eh{h0}")
                if c == 0:
                    units_k = make_proj_units("k", 1) + make_proj_units("q", 1)
                    k_fill = [2, 2, 2, 2, 2, 2, 2, 2]
                elif c < KC - 1:
                    units_k = make_proj_units("k", c + 1)
                    k_fill = [2, 2, 1, 1, 1, 1, 0, 0]
                else:
                    units_k, k_fill = [], [0] * 8
                # --- h0 phase: 8 slots ---
                for kt in range(NT):
                    if kt > 0:
                        scores_slot(h0, kt, eh0)
                    for _ in range(k_fill[kt]):
                        if units_k:
                            units_k.pop(0)()
                    if kt == 0:
                        scores_slot(h0, kt, eh0)
                    # spill: finish attnv of head 2c-2 (qt 6,7)
                    if spill is not None and kt < len(spill[4]):
                        sh, seh, spas, sao, qts = spill
                        attnv_part(sh, qts[kt], seh, spas)
                        if qts[kt] == NT - 1:
                            emit_norm(sh, spas, sao)
                    # attnv of head 2c-1, qt 0..5 on slots 2..7
                    if eh_prev is not None and kt >= 2:
                        if kt == 2:
                            pas_prev = pas_pool.tile([128, N], f32, tag="as",
                                                     name=f"pas{h0 - 1}")
                        attnv_part(h0 - 1, kt - 2, eh_prev, pas_prev)
                spill = None

                emit_ebh(h1 + 2, pebt)
                eh1 = pc.tile([128, NT, N], f16, tag="et", name=f"eh{h1}")
                # --- h1 phase: 8 slots ---
                units_q = make_proj_units("q", c + 1) if 0 < c < KC - 1 else []
                q_fill = [2, 2, 1, 1, 1, 1, 0, 0]
                for kt in range(NT):
                    if kt > 0:
                        scores_slot(h1, kt, eh1)
                    for _ in range(q_fill[kt]):
                        if units_q:
                            units_q.pop(0)()
                    if kt == 0:
                        scores_slot(h1, kt, eh1)
                    if eh_prev is not None and kt < 2:
                        # finish attnv of head 2c-1 (qt 6,7)
                        attnv_part(h0 - 1, 6 + kt, eh_prev, pas_prev)
                        if kt == 1:
                            emit_norm(h0 - 1, pas_prev, ao_last)
                            kq_tiles.pop(("k", c - 1), None)
                            kq_tiles.pop(("q", c - 1), None)
                    if kt == 2 and c >= 1:
                        emit_transpose(c - 1, ao_last)
                    # attnv of head 2c, qt 0..5 on slots 2..7
                    if kt >= 2:
                        if kt == 2:
                            pas0 = pas_pool.tile([128, N], f32, tag="as",
                                                 name=f"pas{h0}")
                        attnv_part(h0, kt - 2, eh0, pas0)
                if c < KC - 1:
                    spill = (h0, eh0, pas0, ao_cur, (6, 7))
                else:
                    for qt in (6, 7):
                        attnv_part(h0, qt, eh0, pas0)
                    emit_norm(h0, pas0, ao_cur)
                eh_prev, pas_prev = eh1, None

            # ---------------- epilogue ----------------
            def oproj_mm(fo, m, nh, kc):
                nc.tensor.matmul(
                    fo[:, nh * 512:(nh + 1) * 512],
                    aoT[:, kc, m * 128:(m + 1) * 128],
                    wo16[:, kc, nh * 512:(nh + 1) * 512],
                    start=(kc == 0), stop=(kc == KC - 1))

            def oproj_out(fo, m, nh):
                so = pc.tile([128, N], f16, tag="so", name=f"so{m}{nh}")
                nh_sl = slice(nh * 512, (nh + 1) * 512)
                nc.vector.tensor_add(out=so[:, nh_sl], in0=fo[:, nh_sl],
                                     in1=bo_b[:, nh_sl])
                nc.sync.dma_start(out=out_d[m * 128:(m + 1) * 128, nh_sl],
                                  in_=so[:, nh_sl])

            # attnv + normalize for head 15; m0's first kc-steps fill the
            # PE while the norm/transpose chain drains (kc=7 deferred)
            pas15 = pas_pool.tile([128, N], f32, tag="as", name="pas15")
            for qt in range(NT):
                attnv_part(H - 1, qt, eh_prev, pas15)
            fo0 = pacc.tile([128, N], f32, tag="acc", name="fo0")
            emit_norm(H - 1, pas15, ao_cur)
            for nh in range(2):
                for kc in range(KC - 1):
                    oproj_mm(fo0, 0, nh, kc)
            emit_transpose(KC - 1, ao_cur)
            for nh in range(2):
                oproj_mm(fo0, 0, nh, KC - 1)
                oproj_out(fo0, 0, nh)

            for m in range(1, NT):
                if m % 2 == 1:
                    fo = pas_pool.tile([128, N], f32, tag="as", name=f"fo{m}")
                else:
                    fo = pacc.tile([128, N], f32, tag="acc", name=f"fo{m}")
                for nh in range(2):
                    for kc in range(KC):
                        oproj_mm(fo, m, nh, kc)
                    oproj_out(fo, m, nh)


def build():
    nc = bacc.Bacc()
    x_in = nc.declare_dram_parameter("x", [N, D], f16, isOutput=False)
    c_in = nc.declare_dram_parameter("ctx", [N, D], f16, isOutput=False)
    wq_in = nc.declare_dram_parameter("wq", [D, D], f16, isOutput=False)
    wk_in = nc.declare_dram_parameter("wk", [D, D], f16, isOutput=False)
    wv_in = nc.declare_dram_parameter("wv", [D, D], f16, isOutput=False)
    wo_in = nc.declare_dram_parameter("wo", [D, D], f16, isOutput=False)
    bqkv_in = nc.declare_dram_parameter("bqkv", [3, D], f32, isOutput=False)
    bo_in = nc.declare_dram_parameter("bo", [D], f32, isOutput=False)
    ebt_in = nc.declare_dram_parameter("ebt", [H, N, N], f16, isOutput=False)
    out_d = nc.declare_dram_parameter("out", [N, D], f16, isOutput=True)
    with tile.TileContext(nc) as tc:
        _body(tc, nc, x_in, c_in, wq_in, wk_in, wv_in, wo_in, bqkv_in, bo_in,
              ebt_in, out_d)
    nc.compile()
    return nc


_NC_CACHE = None


def _get_nc():
    global _NC_CACHE
    if _NC_CACHE is None:
        _NC_CACHE = build()
    return _NC_CACHE


def kernel(x, context, relative_position_bias, Wq, Wk, Wv, Wo, bo, gamma,
           beta):
    x = np.asarray(x, np.float32)
    context = np.asarray(context, np.float32)
    rpb = np.asarray(relative_position_bias, np.float32)
    Wq = np.asarray(Wq, np.float32)
    Wk = np.asarray(Wk, np.float32)
    Wv = np.asarray(Wv, np.float32)
    Wo = np.asarray(Wo, np.float32)
    bo = np.asarray(bo, np.float32)
    gamma = np.asarray(gamma, np.float32)
    beta = np.asarray(beta, np.float32)

    wq16 = (gamma[:, None] * Wq).astype(np.float16)
    wk16 = (gamma[:, None] * Wk).astype(np.float16)
    wv16 = (gamma[:, None] * Wv).astype(np.float16)
    wo16 = Wo.astype(np.float16)
    bqkv = np.stack([beta @ Wq, beta @ Wk, beta @ Wv]).astype(np.float32)
    ebt = np.exp(rpb).transpose(0, 2, 1).astype(np.float16).copy()

    shared = {
        "wq": wq16, "wk": wk16, "wv": wv16, "wo": wo16,
        "bqkv": bqkv, "bo": bo, "ebt": ebt,
    }
    in_maps = [
        {"x": np.ascontiguousarray(x[i]).astype(np.float16),
         "ctx": np.ascontiguousarray(context[i]).astype(np.float16), **shared}
        for i in range(N_CORES)
    ]

    nc = _get_nc()
    last_err = None
    for _attempt in range(3):
        try:
            res = run_bass_kernel_spmd(nc, in_maps, list(range(N_CORES)))
            break
        except Exception as e:  # transient NRT/axon exec errors
            last_err = e
    else:
        raise last_err
    return np.stack([res.results[i]["out"].astype(np.float32)
                     for i in range(N_CORES)])
